# revision 24
# baseline (speedup 1.0000x reference)
"""Trainium2 Bass kernel for the anchor-based NMS matcher (fp16 pipeline).

Math (see problem reference): per (batch b, organ o), over Qp=8192 anchor
queries q:
    cost_class = -sigmoid(logit)
    cost_bbox  = sum_d |anchor_d - tgt_d|            (cxcyczwhd space)
    cost_giou  = -giou3d(xyzxyz(clip(anchor,0)), xyzxyz(tgt))
    C = 5*cb + 2*cc + 2*cg
    matches     = one_hot(argmin_q C) * present
    soft_labels = present ? clip((cg-cgmax)/(cgmin-cgmax), 0) : -1

Device strategy (8 cores, data-parallel over batch, 2 batch items/core):
  SBUF layout: 120 partitions = (organ 20) x (q-chunk 6), free dim N=1366
  (6*1366 = 8196 = 8192 + 4 edge-pad).  All big planes are fp16: DVE
  tensor_scalar runs in 4x mode (416ns/plane) and tensor_tensor in 2x
  mode (772ns) vs 1483ns at fp32, and DMA bytes halve.  Only 9 anchor
  planes are DMA'd (alt/arb endpoints + 2.5x-scaled centers); sizes
  rs = arb - alt, vola, and the scaled size planes are derived on device
  during the DMA-bound startup window.

  negc = sigmoid - 2.5*cb + frac with frac = union/volc + inter/union
  (= giou + 1; affine-invariant for ranking and labels).  Work spread:
    DVE : interval endpoints, geometry, products, frac tail, top-8
    Act : sigmoid, |2.5(a-t)| via Abs, both giou reciprocals as
          exp(-ln(x)) with f32 intermediates (batched Ln then Exp to
          bound act-table reloads), PSUM->SBUF copies
    Pool: size planes, batch-1 p planes, vc2
    PE  : the whole negc sum tree as PSUM-accumulated +/-identity
          matmuls (sig + frac - ab0..ab5), fp16 weights, f32 accum
  Top-8 runs per half-row (copies pipeline into Max), so the device
  emits 16 candidate indices per (b, partition) plus the raw fp16 frac
  plane.  Host (numpy, f64) rescores the 96 candidates per (b,o) with
  the exact reference formula for exact argmin matches, and computes
  soft labels as the affine normalization of frac (absent organs
  patched to -1 on host).  Validated on the seeded data: full candidate
  coverage, label l2 rel err ~7e-4.
"""

import numpy as np

import concourse.bacc as bacc
import concourse.bass as bass
import concourse.mybir as mybir
from concourse.bass_utils import run_bass_kernel_spmd
from concourse.tile import TileContext

F32 = mybir.dt.float32
F16 = mybir.dt.float16
U16 = mybir.dt.uint16
U32 = mybir.dt.uint32
ALU = mybir.AluOpType
ACTF = mybir.ActivationFunctionType
AXL = mybir.AxisListType

BS, O, QP = 16, 20, 8192
NCORES = 8
BL = BS // NCORES        # batch items per core
NCH = 6                  # q chunks per organ
N = 1366                 # chunk width; 6*1366 = 8196 = 8192 + 4 pad
P = O * NCH              # 120 partitions
NPLANES = 9              # alt0,arb0,alt1,arb1,alt2,arb2, ap0-2 (2.5x ctr)
MMW = 512                # matmul moving free-dim chunk
MMC = [(0, 512), (512, 1024), (1024, 1366)]
MXS = [(0, 512), (512, 1366)]   # Max/MaxIndex slices

# sc column indices (per-partition f32 scalars, per batch item)
C_BLT = 0    # 0..2
C_BRB = 3    # 3..5
C_FD = 6     # 6..8
C_VOLB = 9
C_TQ = 10    # 10..15  (2.5 * target comps)
NSC = 16

_BUILT = {}


def _build_nc():
    nc = bacc.Bacc("TRN2", target_bir_lowering=False, debug=False)
    ath = nc.dram_tensor("ath", [NPLANES, P, N], F16, kind="ExternalInput")
    lg = nc.dram_tensor("lg", [BL, P, N], F16, kind="ExternalInput")
    sc = nc.dram_tensor("sc", [BL, P, NSC], F32, kind="ExternalInput")
    fr = nc.dram_tensor("fr", [BL, P, N], F16, kind="ExternalOutput")
    ixo = nc.dram_tensor("ixo", [BL, P, 16], U32, kind="ExternalOutput")

    from concourse.masks import make_identity

    with TileContext(nc) as tc:
        with (
            tc.tile_pool(name="big", bufs=1) as big,
            tc.tile_pool(name="sm", bufs=1) as sm,
            tc.tile_pool(name="ps", bufs=1, space="PSUM") as ps,
        ):
            # ---------------- small consts ----------------
            sct = sm.tile([P, BL, NSC], F32, tag="sct", name="sct")
            nc.scalar.dma_start(out=sct[:], in_=sc.rearrange("b p i -> p b i"))

            def col(b, i):
                return sct[:, b, i : i + 1]

            ipos = sm.tile([P, P], F16, tag="ipos", name="ipos")
            make_identity(nc, ipos[:])
            ineg = sm.tile([P, P], F16, tag="ineg", name="ineg")
            nc.gpsimd.memset(ineg[:], 0.0)
            nc.gpsimd.affine_select(
                out=ineg[:], in_=ineg[:],
                compare_op=ALU.not_equal, fill=-1.0, base=0,
                pattern=[[-1, P]], channel_multiplier=1)

            # ---------------- big inputs ----------------
            lgt = [big.tile([P, N], F16, tag=f"lg{b}", name=f"lg{b}")
                   for b in range(BL)]
            ain = big.tile([P, NPLANES, N], F16, tag="ain", name="ain")

            def v(j):
                return ain[:, j, :]

            ALT = [v(0), v(2), v(4)]
            ARB = [v(1), v(3), v(5)]
            APC = [v(6), v(7), v(8)]          # 2.5x center comps

            def load_planes(j0, j1):
                nc.sync.dma_start(out=ain[:, j0:j1, :],
                                  in_=ath[j0:j1].rearrange("i p n -> p i n"))

            load_planes(0, 2)           # alt0, arb0
            for b in range(BL):
                nc.scalar.dma_start(out=lgt[b][:], in_=lg[b])
            load_planes(2, 4)           # alt1, arb1
            load_planes(4, 6)           # alt2, arb2
            load_planes(6, 7)           # ap0
            load_planes(7, 8)           # ap1
            load_planes(8, 9)           # ap2

            # ---------------- per-batch tiles ----------------
            def bt(tag, b, dt=F16):
                return big.tile([P, N], dt, tag=f"{tag}{b}", name=f"{tag}{b}")

            RSD = [big.tile([P, N], F16, tag=f"rs{d}", name=f"rs{d}")
                   for d in range(3)]
            VOLA = big.tile([P, N], F16, tag="vola", name="vola")
            SIG = [bt("sig", b) for b in range(BL)]
            MX = [[bt(f"mx{d}_", b) for d in range(3)] for b in range(BL)]
            MN = [[bt(f"mn{d}_", b) for d in range(3)] for b in range(BL)]
            PD = [[bt(f"pd{d}_", b) for d in range(3)] for b in range(BL)]
            SF = [[bt(f"sf{d}_", b) for d in range(3)] for b in range(BL)]
            MM = [[bt(f"mm{d}_", b) for d in range(3)] for b in range(BL)]
            INT = [bt("int", b) for b in range(BL)]
            VLC = [bt("vlc", b) for b in range(BL)]
            UNI = [bt("uni", b) for b in range(BL)]
            RU = [bt("ru", b) for b in range(BL)]
            RV = [bt("rv", b) for b in range(BL)]
            LND = big.tile([P, N, 4], F32, tag="lnd", name="lnd")
            FRC = [bt("frc", b) for b in range(BL)]
            NG16 = [bt("ng16", b) for b in range(BL)]
            # SBUF reuse: abs planes land in MX/MN (dead after p_d);
            # vc_d lands in SF (dead after vc); XT lands in MM[0].
            VC = SF
            AB = [[MX[b][0], MX[b][1], MX[b][2],
                   MN[b][0], MN[b][1], MN[b][2]] for b in range(BL)]
            XT = [MM[b][0] for b in range(BL)]
            NGP = [ps.tile([P, 3, MMW], F32, tag=f"ngp{b}", name=f"ngp{b}")
                   for b in range(BL)]

            def mm_acc(b, plane, wt, start, stop, chunks=MMC):
                for c0, c1 in chunks:
                    c = MMC.index((c0, c1))
                    nc.tensor.matmul(NGP[b][:, c, : c1 - c0], wt[:],
                                     plane[:, c0:c1], start=start, stop=stop)

            # ---------------- geometry ----------------
            # per-d: endpoints (DVE), rs (Pool), p (DVE b0 / Pool b1).
            # Union side first (it feeds the first Ln), volc side after.
            for d in range(3):
                for b in range(BL):
                    nc.vector.tensor_scalar_max(out=MX[b][d][:], in0=ALT[d],
                                                scalar1=col(b, C_BLT + d))
                    nc.vector.tensor_scalar_min(out=MN[b][d][:], in0=ARB[d],
                                                scalar1=col(b, C_BRB + d))
                nc.gpsimd.tensor_tensor(out=RSD[d][:], in0=ARB[d],
                                        in1=ALT[d], op=ALU.subtract)
                nc.vector.tensor_tensor(out=PD[0][d][:], in0=MN[0][d][:],
                                        in1=MX[0][d][:], op=ALU.subtract)
                nc.gpsimd.tensor_tensor(out=PD[1][d][:], in0=MN[1][d][:],
                                        in1=MX[1][d][:], op=ALU.subtract)
                if d == 0:
                    for b in range(BL):
                        nc.scalar.activation(SIG[b][:], lgt[b][:],
                                             ACTF.Sigmoid)
                if d == 1:
                    for b in range(BL):
                        nc.scalar.activation(AB[b][0][:], APC[0], ACTF.Abs,
                                             bias=col(b, C_TQ), scale=-2.5)
                if d == 2:
                    for b in range(BL):
                        nc.scalar.activation(AB[b][1][:], APC[1], ACTF.Abs,
                                             bias=col(b, C_TQ + 1),
                                             scale=-2.5)
            # Interleaved union/volc chains so both recip inputs land
            # together, just as Act drains the abs block.
            for b in range(BL):
                nc.vector.tensor_scalar_add(out=SF[b][0][:], in0=RSD[0][:],
                                            scalar1=col(b, C_FD))
                nc.vector.tensor_scalar_max(out=MM[b][0][:],
                                            in0=PD[b][0][:], scalar1=0.0)
            for b in range(BL):
                nc.vector.tensor_tensor(out=VC[b][0][:], in0=SF[b][0][:],
                                        in1=PD[b][0][:], op=ALU.subtract)
                nc.vector.tensor_scalar_max(out=MM[b][1][:],
                                            in0=PD[b][1][:], scalar1=0.0)
            for b in range(BL):
                nc.vector.tensor_scalar_add(out=SF[b][1][:], in0=RSD[1][:],
                                            scalar1=col(b, C_FD + 1))
                nc.vector.tensor_tensor(out=XT[b][:], in0=MM[b][0][:],
                                        in1=MM[b][1][:], op=ALU.mult)
            for b in range(BL):
                # vc1 on Pool so the volc product can close early
                nc.gpsimd.tensor_tensor(out=VC[b][1][:], in0=SF[b][1][:],
                                        in1=PD[b][1][:], op=ALU.subtract)
                nc.vector.tensor_scalar_max(out=MM[b][2][:],
                                            in0=PD[b][2][:], scalar1=0.0)
            for b in range(BL):
                nc.vector.tensor_tensor(out=INT[b][:], in0=XT[b][:],
                                        in1=MM[b][2][:], op=ALU.mult)
                nc.vector.tensor_scalar_add(out=SF[b][2][:], in0=RSD[2][:],
                                            scalar1=col(b, C_FD + 2))
            nc.vector.tensor_tensor(out=VOLA[:], in0=RSD[0][:],
                                    in1=RSD[1][:], op=ALU.mult)
            nc.vector.tensor_tensor(out=VOLA[:], in0=VOLA[:],
                                    in1=RSD[2][:], op=ALU.mult)
            for b in range(BL):
                nc.vector.tensor_tensor(out=VC[b][2][:], in0=SF[b][2][:],
                                        in1=PD[b][2][:], op=ALU.subtract)
                nc.vector.tensor_scalar_add(out=UNI[b][:], in0=VOLA[:],
                                            scalar1=col(b, C_VOLB))
            for b in range(BL):
                nc.vector.tensor_tensor(out=VLC[b][:], in0=VC[b][0][:],
                                        in1=VC[b][1][:], op=ALU.mult)
                nc.vector.tensor_tensor(out=UNI[b][:], in0=UNI[b][:],
                                        in1=INT[b][:], op=ALU.subtract)
            for b in range(BL):
                nc.vector.tensor_tensor(out=VLC[b][:], in0=VLC[b][:],
                                        in1=VC[b][2][:], op=ALU.mult)
            for b in range(BL):
                nc.scalar.activation(AB[b][2][:], APC[2], ACTF.Abs,
                                     bias=col(b, C_TQ + 2), scale=-2.5)
                nc.scalar.activation(AB[b][3][:], RSD[0][:], ACTF.Abs,
                                     bias=col(b, C_TQ + 3), scale=-2.5)
            for b in range(BL):
                nc.scalar.activation(AB[b][4][:], RSD[1][:], ACTF.Abs,
                                     bias=col(b, C_TQ + 4), scale=-2.5)
                nc.scalar.activation(AB[b][5][:], RSD[2][:], ACTF.Abs,
                                     bias=col(b, C_TQ + 5), scale=-2.5)
            # PE: negc accumulation for all early planes
            for b in range(BL):
                mm_acc(b, SIG[b], ipos, True, False)
                mm_acc(b, AB[b][0], ineg, False, False)
                mm_acc(b, AB[b][1], ineg, False, False)
                mm_acc(b, AB[b][2], ineg, False, False)
                mm_acc(b, AB[b][3], ineg, False, False)
                mm_acc(b, AB[b][4], ineg, False, False)
                mm_acc(b, AB[b][5], ineg, False, False)
            # Act: Ln block in readiness order (u first), then Exps
            for b in range(BL):
                nc.scalar.activation(LND[:, :, b], UNI[b][:], ACTF.Ln)
            for b in range(BL):
                nc.scalar.activation(LND[:, :, 2 + b], VLC[b][:], ACTF.Ln)
            nc.scalar.activation(RU[0][:], LND[:, :, 0], ACTF.Exp, scale=-1.0)
            nc.scalar.activation(RV[0][:], LND[:, :, 2], ACTF.Exp, scale=-1.0)
            nc.scalar.activation(RU[1][:], LND[:, :, 1], ACTF.Exp, scale=-1.0)
            nc.scalar.activation(RV[1][:], LND[:, :, 3], ACTF.Exp, scale=-1.0)
            # DVE tail: frac = union*rv + inter*ru, per-chunk PE accumulate
            # finish, Act copies, sliced top-8 (pipelined per batch)
            mx8 = [[sm.tile([P, 8], F16, tag=f"mx8_{b}_{s}",
                            name=f"mx8_{b}_{s}") for s in range(2)]
                   for b in range(BL)]
            ix8 = [sm.tile([P, 16], U32, tag=f"ix8_{b}", name=f"ix8_{b}")
                   for b in range(BL)]
            for b in range(BL):
                nc.vector.tensor_tensor(out=XT[b][:], in0=UNI[b][:],
                                        in1=RV[b][:], op=ALU.mult)
                nc.vector.tensor_tensor(out=FRC[b][:], in0=INT[b][:],
                                        in1=RU[b][:], op=ALU.mult)
                nc.vector.tensor_tensor(out=FRC[b][:], in0=FRC[b][:],
                                        in1=XT[b][:], op=ALU.add)
                nc.sync.dma_start(out=fr[b], in_=FRC[b][:])
                mm_acc(b, FRC[b], ipos, False, True)
                for c, (c0, c1) in enumerate(MMC):
                    nc.scalar.activation(NG16[b][:, c0:c1],
                                         NGP[b][:, c, : c1 - c0], ACTF.Copy)
                for s, (s0, s1) in enumerate(MXS):
                    nc.vector.max(out=mx8[b][s][:], in_=NG16[b][:, s0:s1])
                    nc.vector.max_index(out=ix8[b][:, 8 * s : 8 * s + 8],
                                        in_max=mx8[b][s][:],
                                        in_values=NG16[b][:, s0:s1])
                nc.sync.dma_start(out=ixo[b], in_=ix8[b][:])

    nc.finalize()
    return nc


def _prep_host(pred_logits, anchors, target_boxes, target_present):
    f32, f16 = np.float32, np.float16
    A = np.ascontiguousarray(anchors.reshape(O, QP, 6).astype(f32, copy=False))
    pad = lambda x: np.pad(x, ((0, 0), (0, NCH * N - QP)), mode="edge")

    rc = [np.maximum(A[:, :, d], f32(0)) for d in range(3)]
    rsz = [np.maximum(A[:, :, 3 + d], f32(0)) for d in range(3)]
    alt = [pad(rc[d] - f32(0.5) * rsz[d]) for d in range(3)]
    arb = [pad(rc[d] + f32(0.5) * rsz[d]) for d in range(3)]
    apc = [pad(A[:, :, d]) for d in range(3)]
    planes = [alt[0], arb[0], alt[1], arb[1], alt[2], arb[2]] + apc
    ath = np.stack([p.reshape(P, N) for p in planes]).astype(f16)
    ath = np.ascontiguousarray(ath)

    lgs = pred_logits.reshape(BS, O, QP).astype(f32, copy=False)
    lgs = np.pad(lgs, ((0, 0), (0, 0), (0, NCH * N - QP)), mode="edge")
    lg_all = np.ascontiguousarray(lgs.reshape(BS, P, N).astype(f16))

    t = target_boxes.astype(f32, copy=False)          # [BS, O, 6]
    tc_, ts_ = t[..., :3], t[..., 3:]
    blt = tc_ - f32(0.5) * ts_
    brb = tc_ + f32(0.5) * ts_
    fd = brb - blt
    volb = (fd[..., 0] * fd[..., 1]) * fd[..., 2]

    in_maps = []
    for c in range(NCORES):
        b0 = c * BL
        scv = np.zeros((BL, P, NSC), f32)
        sc3 = scv.reshape(BL, O, NCH, NSC)
        for b in range(BL):
            gb = b0 + b
            sc3[b, :, :, C_BLT : C_BLT + 3] = blt[gb][:, None, :]
            sc3[b, :, :, C_BRB : C_BRB + 3] = brb[gb][:, None, :]
            sc3[b, :, :, C_FD : C_FD + 3] = fd[gb][:, None, :]
            sc3[b, :, :, C_VOLB] = volb[gb][:, None]
            sc3[b, :, :, C_TQ : C_TQ + 6] = f32(2.5) * t[gb][:, None, :]
        in_maps.append({"ath": ath,
                        "lg": np.ascontiguousarray(lg_all[b0 : b0 + BL]),
                        "sc": scv})
    return in_maps


def _host_post(res_results, pred_logits, anchors, target_boxes,
               target_present):
    """Exact matches via f64 rescore of device candidates; labels from
    the fp16 frac planes."""
    f64 = np.float64
    A = anchors.astype(f64).reshape(O, QP, 6)
    pl = pred_logits.astype(f64).reshape(BS, O, QP)
    t = target_boxes.astype(f64)
    present = target_present.astype(bool)

    # candidate q indices: per (b, organ, chunk) two 8-cand slices with
    # free-dim offsets 0 and 512
    K = NCH * 16
    cand = np.empty((BS, O, K), np.int64)
    frac = np.empty((BS, O, QP), np.float16)
    soff = np.array([s0 for s0, _ in MXS], np.int64)
    for c, r in enumerate(res_results):
        b0 = c * BL
        ix = r["ixo"].astype(np.int64).reshape(BL, O, NCH, 2, 8)
        gq = (ix + soff[None, None, None, :, None]
              + (np.arange(NCH, dtype=np.int64) * N)[None, None, :, None,
                                                     None])
        np.clip(gq, 0, QP - 1, out=gq)
        cand[b0 : b0 + BL] = gq.reshape(BL, O, K)
        frac[b0 : b0 + BL] = r["fr"].reshape(BL, O, NCH * N)[:, :, :QP]

    # f64 rescore with the exact reference formula
    bidx = np.arange(BS)[:, None, None]
    oidx = np.arange(O)[None, :, None]
    ab = A[oidx, cand]                                # [BS, O, K, 6]
    lgc = pl[bidx, oidx, cand]                        # [BS, O, K]
    tgt = t[:, :, None, :]                            # [BS, O, 1, 6]
    cb = np.abs(ab - tgt).sum(-1)
    cc = -1.0 / (1.0 + np.exp(-lgc))
    abx = np.clip(ab, 0.0, None)
    a_lt = abx[..., :3] - 0.5 * abx[..., 3:]
    a_rb = abx[..., :3] + 0.5 * abx[..., 3:]
    b_lt = tgt[..., :3] - 0.5 * tgt[..., 3:]
    b_rb = tgt[..., :3] + 0.5 * tgt[..., 3:]
    va = np.prod(a_rb - a_lt, -1)
    vb = np.prod(b_rb - b_lt, -1)
    it = np.prod(np.clip(np.minimum(a_rb, b_rb) - np.maximum(a_lt, b_lt),
                         0.0, None), -1)
    un = va + vb - it
    vcb = np.prod(np.clip(np.maximum(a_rb, b_rb) - np.minimum(a_lt, b_lt),
                          0.0, None), -1)
    giou = it / un - (vcb - un) / vcb
    Cc = 5.0 * cb + 2.0 * cc + 2.0 * (-giou)
    # argmin with lowest-q tie-break (reference top_k picks first index)
    order = np.lexsort((cand, Cc), axis=-1)
    best = np.take_along_axis(cand, order[..., :1], axis=-1)[..., 0]

    matches = np.zeros((BS, O, QP), np.int32)
    bo_b, bo_o = np.nonzero(present)
    matches[bo_b, bo_o, best[bo_b, bo_o]] = 1

    f = frac.astype(f64)
    fmin = f.min(-1, keepdims=True)
    fmax = f.max(-1, keepdims=True)
    sl = np.clip((f - fmin) / (fmax - fmin), 0.0, None).astype(np.float32)
    soft = np.where(present[..., None], sl, np.float32(-1.0))
    return matches, soft


def kernel(pred_logits, pred_boxes, anchors, target_boxes, target_present,
           num_top_queries):
    k = int(num_top_queries)
    assert k == 1, f"kernel specialized for num_top_queries=1, got {k}"

    if "nc" not in _BUILT:
        _BUILT["nc"] = _build_nc()
    nc = _BUILT["nc"]

    pred_logits = np.asarray(pred_logits)
    anchors = np.asarray(anchors)
    target_boxes = np.asarray(target_boxes)
    target_present = np.asarray(target_present)
    in_maps = _prep_host(pred_logits, anchors, target_boxes, target_present)
    res = run_bass_kernel_spmd(nc, in_maps, core_ids=list(range(NCORES)))
    return _host_post(res.results, pred_logits, anchors, target_boxes,
                      target_present)


# revision 29
# speedup vs baseline: 1.0123x; 1.0123x over previous
"""Trainium2 Bass kernel for the anchor-based NMS matcher (fp16 pipeline).

Math (see problem reference): per (batch b, organ o), over Qp=8192 anchor
queries q:
    cost_class = -sigmoid(logit)
    cost_bbox  = sum_d |anchor_d - tgt_d|            (cxcyczwhd space)
    cost_giou  = -giou3d(xyzxyz(clip(anchor,0)), xyzxyz(tgt))
    C = 5*cb + 2*cc + 2*cg
    matches     = one_hot(argmin_q C) * present
    soft_labels = present ? clip((cg-cgmax)/(cgmin-cgmax), 0) : -1

Device strategy (8 cores, data-parallel over batch, 2 batch items/core):
  SBUF layout: 120 partitions = (organ 20) x (q-chunk 6), free dim N=1366
  (6*1366 = 8196 = 8192 + 4 edge-pad).  All big planes are fp16: DVE
  tensor_scalar runs in 4x mode (416ns/plane) and tensor_tensor in 2x
  mode (772ns) vs 1483ns at fp32, and DMA bytes halve.  Only 9 anchor
  planes are DMA'd (alt/arb endpoints + 2.5x-scaled centers); sizes
  rs = arb - alt, vola, and the scaled size planes are derived on device
  during the DMA-bound startup window.

  negc = sigmoid - 2.5*cb + frac with frac = union/volc + inter/union
  (= giou + 1; affine-invariant for ranking and labels).  Work spread:
    DVE : interval endpoints, geometry, products, frac tail, top-8
    Act : sigmoid, |2.5(a-t)| via Abs, both giou reciprocals as
          exp(-ln(x)) with f32 intermediates (batched Ln then Exp to
          bound act-table reloads), PSUM->SBUF copies
    Pool: size planes, batch-1 p planes, vc2
    PE  : the whole negc sum tree as PSUM-accumulated +/-identity
          matmuls (sig + frac - ab0..ab5), fp16 weights, f32 accum
  Top-8 runs per half-row (copies pipeline into Max), so the device
  emits 16 candidate indices per (b, partition) plus the raw fp16 frac
  plane.  Host (numpy, f64) rescores the 96 candidates per (b,o) with
  the exact reference formula for exact argmin matches, and computes
  soft labels as the affine normalization of frac (absent organs
  patched to -1 on host).  Validated on the seeded data: full candidate
  coverage, label l2 rel err ~7e-4.
"""

import numpy as np

import concourse.bacc as bacc
import concourse.bass as bass
import concourse.mybir as mybir
from concourse.bass_utils import run_bass_kernel_spmd
from concourse.tile import TileContext

F32 = mybir.dt.float32
F16 = mybir.dt.float16
U16 = mybir.dt.uint16
U32 = mybir.dt.uint32
ALU = mybir.AluOpType
ACTF = mybir.ActivationFunctionType
AXL = mybir.AxisListType

BS, O, QP = 16, 20, 8192
NCORES = 8
BL = BS // NCORES        # batch items per core
NCH = 6                  # q chunks per organ
N = 1366                 # chunk width; 6*1366 = 8196 = 8192 + 4 pad
P = O * NCH              # 120 partitions
NPLANES = 9              # alt0,arb0,alt1,arb1,alt2,arb2, ap0-2 (2.5x ctr)
MMW = 512                # matmul moving free-dim chunk
MMC = [(0, 512), (512, 1024), (1024, 1366)]
MXS = [(0, 512), (512, 1366)]   # Max/MaxIndex slices

# sc column indices (per-partition f32 scalars, per batch item)
C_BLT = 0    # 0..2
C_BRB = 3    # 3..5
C_FD = 6     # 6..8
C_VOLB = 9
C_TQ = 10    # 10..15  (2.5 * target comps)
NSC = 16

_BUILT = {}


def _build_nc():
    nc = bacc.Bacc("TRN2", target_bir_lowering=False, debug=False)
    ath = nc.dram_tensor("ath", [NPLANES, P, N], F16, kind="ExternalInput")
    lg = nc.dram_tensor("lg", [BL, P, N], F16, kind="ExternalInput")
    sc = nc.dram_tensor("sc", [BL, P, NSC], F32, kind="ExternalInput")
    fr = nc.dram_tensor("fr", [BL, P, N], F16, kind="ExternalOutput")
    ixo = nc.dram_tensor("ixo", [BL, P, 16], U32, kind="ExternalOutput")

    from concourse.masks import make_identity

    with TileContext(nc) as tc:
        with (
            tc.tile_pool(name="big", bufs=1) as big,
            tc.tile_pool(name="sm", bufs=1) as sm,
            tc.tile_pool(name="ps", bufs=1, space="PSUM") as ps,
        ):
            # ---------------- small consts ----------------
            sct = sm.tile([P, BL, NSC], F32, tag="sct", name="sct")
            nc.scalar.dma_start(out=sct[:], in_=sc.rearrange("b p i -> p b i"))

            def col(b, i):
                return sct[:, b, i : i + 1]

            ipos = sm.tile([P, P], F16, tag="ipos", name="ipos")
            make_identity(nc, ipos[:])
            ineg = sm.tile([P, P], F16, tag="ineg", name="ineg")
            nc.gpsimd.memset(ineg[:], 0.0)
            nc.gpsimd.affine_select(
                out=ineg[:], in_=ineg[:],
                compare_op=ALU.not_equal, fill=-1.0, base=0,
                pattern=[[-1, P]], channel_multiplier=1)

            # ---------------- big inputs ----------------
            lgt = [big.tile([P, N], F16, tag=f"lg{b}", name=f"lg{b}")
                   for b in range(BL)]
            ain = big.tile([P, NPLANES, N], F16, tag="ain", name="ain")

            def v(j):
                return ain[:, j, :]

            ALT = [v(0), v(2), v(4)]
            ARB = [v(1), v(3), v(5)]
            APC = [v(6), v(7), v(8)]          # 2.5x center comps

            def load_planes(j0, j1):
                nc.sync.dma_start(out=ain[:, j0:j1, :],
                                  in_=ath[j0:j1].rearrange("i p n -> p i n"))

            load_planes(0, 2)           # alt0, arb0
            for b in range(BL):
                nc.scalar.dma_start(out=lgt[b][:], in_=lg[b])
            load_planes(2, 4)           # alt1, arb1
            load_planes(4, 6)           # alt2, arb2
            load_planes(6, 7)           # ap0
            load_planes(7, 8)           # ap1
            load_planes(8, 9)           # ap2

            # ---------------- per-batch tiles ----------------
            def bt(tag, b, dt=F16):
                return big.tile([P, N], dt, tag=f"{tag}{b}", name=f"{tag}{b}")

            RSD = [big.tile([P, N], F16, tag=f"rs{d}", name=f"rs{d}")
                   for d in range(3)]
            VOLA = big.tile([P, N], F16, tag="vola", name="vola")
            SIG = [bt("sig", b) for b in range(BL)]
            MX = [[bt(f"mx{d}_", b) for d in range(3)] for b in range(BL)]
            MN = [[bt(f"mn{d}_", b) for d in range(3)] for b in range(BL)]
            PD = [[bt(f"pd{d}_", b) for d in range(3)] for b in range(BL)]
            SF = [[bt(f"sf{d}_", b) for d in range(3)] for b in range(BL)]
            MM = [[bt(f"mm{d}_", b) for d in range(3)] for b in range(BL)]
            INT = [bt("int", b) for b in range(BL)]
            VLC = [bt("vlc", b) for b in range(BL)]
            UNI = [bt("uni", b) for b in range(BL)]
            RU = [bt("ru", b) for b in range(BL)]
            RV = [bt("rv", b) for b in range(BL)]
            LND = big.tile([P, N, 4], F32, tag="lnd", name="lnd")
            FRC = [bt("frc", b) for b in range(BL)]
            NG16 = [bt("ng16", b) for b in range(BL)]
            # SBUF reuse: abs planes land in MX/MN (dead after p_d);
            # vc_d lands in SF (dead after vc); XT lands in MM[0].
            VC = SF
            AB = [[MX[b][0], MX[b][1], MX[b][2],
                   MN[b][0], MN[b][1], MN[b][2]] for b in range(BL)]
            XT = [MM[b][0] for b in range(BL)]
            NGP = [ps.tile([P, 3, MMW], F32, tag=f"ngp{b}", name=f"ngp{b}")
                   for b in range(BL)]

            def mm_acc(b, plane, wt, start, stop, chunks=MMC):
                for c0, c1 in chunks:
                    c = MMC.index((c0, c1))
                    nc.tensor.matmul(NGP[b][:, c, : c1 - c0], wt[:],
                                     plane[:, c0:c1], start=start, stop=stop)

            # ---------------- geometry ----------------
            # per-d: endpoints (DVE), rs (Pool), p (DVE b0 / Pool b1).
            # Union side first (it feeds the first Ln), volc side after.
            for d in range(3):
                for b in range(BL):
                    nc.vector.tensor_scalar_max(out=MX[b][d][:], in0=ALT[d],
                                                scalar1=col(b, C_BLT + d))
                    nc.vector.tensor_scalar_min(out=MN[b][d][:], in0=ARB[d],
                                                scalar1=col(b, C_BRB + d))
                nc.gpsimd.tensor_tensor(out=RSD[d][:], in0=ARB[d],
                                        in1=ALT[d], op=ALU.subtract)
                nc.vector.tensor_tensor(out=PD[0][d][:], in0=MN[0][d][:],
                                        in1=MX[0][d][:], op=ALU.subtract)
                nc.gpsimd.tensor_tensor(out=PD[1][d][:], in0=MN[1][d][:],
                                        in1=MX[1][d][:], op=ALU.subtract)
                if d == 0:
                    for b in range(BL):
                        nc.scalar.activation(SIG[b][:], lgt[b][:],
                                             ACTF.Sigmoid)
                if d == 1:
                    for b in range(BL):
                        nc.scalar.activation(AB[b][0][:], APC[0], ACTF.Abs,
                                             bias=col(b, C_TQ), scale=-2.5)
                if d == 2:
                    for b in range(BL):
                        nc.scalar.activation(AB[b][1][:], APC[1], ACTF.Abs,
                                             bias=col(b, C_TQ + 1),
                                             scale=-2.5)
            # Interleaved union/volc chains so both recip inputs land
            # together, just as Act drains the abs block.
            for b in range(BL):
                nc.vector.tensor_scalar_add(out=SF[b][0][:], in0=RSD[0][:],
                                            scalar1=col(b, C_FD))
                nc.vector.tensor_scalar_max(out=MM[b][0][:],
                                            in0=PD[b][0][:], scalar1=0.0)
            for b in range(BL):
                nc.vector.tensor_tensor(out=VC[b][0][:], in0=SF[b][0][:],
                                        in1=PD[b][0][:], op=ALU.subtract)
                nc.vector.tensor_scalar_max(out=MM[b][1][:],
                                            in0=PD[b][1][:], scalar1=0.0)
            for b in range(BL):
                nc.vector.tensor_scalar_add(out=SF[b][1][:], in0=RSD[1][:],
                                            scalar1=col(b, C_FD + 1))
                nc.vector.tensor_tensor(out=XT[b][:], in0=MM[b][0][:],
                                        in1=MM[b][1][:], op=ALU.mult)
            for b in range(BL):
                # vc1 on Pool so the volc product can close early
                nc.gpsimd.tensor_tensor(out=VC[b][1][:], in0=SF[b][1][:],
                                        in1=PD[b][1][:], op=ALU.subtract)
                nc.vector.tensor_scalar_max(out=MM[b][2][:],
                                            in0=PD[b][2][:], scalar1=0.0)
            for b in range(BL):
                nc.vector.tensor_tensor(out=INT[b][:], in0=XT[b][:],
                                        in1=MM[b][2][:], op=ALU.mult)
                nc.vector.tensor_scalar_add(out=SF[b][2][:], in0=RSD[2][:],
                                            scalar1=col(b, C_FD + 2))
            nc.vector.tensor_tensor(out=VOLA[:], in0=RSD[0][:],
                                    in1=RSD[1][:], op=ALU.mult)
            nc.vector.tensor_tensor(out=VOLA[:], in0=VOLA[:],
                                    in1=RSD[2][:], op=ALU.mult)
            for b in range(BL):
                nc.vector.tensor_tensor(out=VC[b][2][:], in0=SF[b][2][:],
                                        in1=PD[b][2][:], op=ALU.subtract)
                nc.vector.tensor_scalar_add(out=UNI[b][:], in0=VOLA[:],
                                            scalar1=col(b, C_VOLB))
            for b in range(BL):
                nc.vector.tensor_tensor(out=VLC[b][:], in0=VC[b][0][:],
                                        in1=VC[b][1][:], op=ALU.mult)
                nc.vector.tensor_tensor(out=UNI[b][:], in0=UNI[b][:],
                                        in1=INT[b][:], op=ALU.subtract)
            for b in range(BL):
                nc.vector.tensor_tensor(out=VLC[b][:], in0=VLC[b][:],
                                        in1=VC[b][2][:], op=ALU.mult)
            for b in range(BL):
                nc.scalar.activation(AB[b][4][:], RSD[1][:], ACTF.Abs,
                                     bias=col(b, C_TQ + 4), scale=-2.5)
                nc.scalar.activation(AB[b][5][:], RSD[2][:], ACTF.Abs,
                                     bias=col(b, C_TQ + 5), scale=-2.5)
            # PE: negc accumulation for the early planes
            for b in range(BL):
                mm_acc(b, SIG[b], ipos, True, False)
                mm_acc(b, AB[b][0], ineg, False, False)
                mm_acc(b, AB[b][1], ineg, False, False)
                mm_acc(b, AB[b][4], ineg, False, False)
                mm_acc(b, AB[b][5], ineg, False, False)
            # Act: Ln block in readiness order (u first), then Exps
            for b in range(BL):
                nc.scalar.activation(LND[:, :, b], UNI[b][:], ACTF.Ln)
            for b in range(BL):
                nc.scalar.activation(LND[:, :, 2 + b], VLC[b][:], ACTF.Ln)
            nc.scalar.activation(RU[0][:], LND[:, :, 0], ACTF.Exp, scale=-1.0)
            nc.scalar.activation(RV[0][:], LND[:, :, 2], ACTF.Exp, scale=-1.0)
            nc.scalar.activation(RU[1][:], LND[:, :, 1], ACTF.Exp, scale=-1.0)
            nc.scalar.activation(RV[1][:], LND[:, :, 3], ACTF.Exp, scale=-1.0)
            # DVE abs pairs for ab2/ab3 fill the recip-wait hole
            for b in range(BL):
                nc.vector.tensor_scalar(out=AB[b][2][:], in0=APC[2],
                                        scalar1=2.5, scalar2=col(b, C_TQ + 2),
                                        op0=ALU.mult, op1=ALU.subtract)
                nc.vector.tensor_scalar(out=AB[b][2][:].bitcast(U16),
                                        in0=AB[b][2][:].bitcast(U16),
                                        scalar1=0x7FFF, scalar2=None,
                                        op0=ALU.bitwise_and)
                nc.vector.tensor_scalar(out=AB[b][3][:], in0=RSD[0][:],
                                        scalar1=2.5, scalar2=col(b, C_TQ + 3),
                                        op0=ALU.mult, op1=ALU.subtract)
                nc.vector.tensor_scalar(out=AB[b][3][:].bitcast(U16),
                                        in0=AB[b][3][:].bitcast(U16),
                                        scalar1=0x7FFF, scalar2=None,
                                        op0=ALU.bitwise_and)
            for b in range(BL):
                mm_acc(b, AB[b][2], ineg, False, False)
                mm_acc(b, AB[b][3], ineg, False, False)
            # DVE tail: frac = union*rv + inter*ru, per-chunk PE accumulate
            # finish, Act copies, sliced top-8 (pipelined per batch)
            mx8 = [[sm.tile([P, 8], F16, tag=f"mx8_{b}_{s}",
                            name=f"mx8_{b}_{s}") for s in range(2)]
                   for b in range(BL)]
            ix8 = [sm.tile([P, 16], U32, tag=f"ix8_{b}", name=f"ix8_{b}")
                   for b in range(BL)]
            for b in range(BL):
                nc.vector.tensor_tensor(out=XT[b][:], in0=UNI[b][:],
                                        in1=RV[b][:], op=ALU.mult)
                nc.vector.tensor_tensor(out=FRC[b][:], in0=INT[b][:],
                                        in1=RU[b][:], op=ALU.mult)
                nc.vector.tensor_tensor(out=FRC[b][:], in0=FRC[b][:],
                                        in1=XT[b][:], op=ALU.add)
                nc.sync.dma_start(out=fr[b], in_=FRC[b][:])
                mm_acc(b, FRC[b], ipos, False, True)
                for c, (c0, c1) in enumerate(MMC):
                    nc.scalar.activation(NG16[b][:, c0:c1],
                                         NGP[b][:, c, : c1 - c0], ACTF.Copy)
                for s, (s0, s1) in enumerate(MXS):
                    nc.vector.max(out=mx8[b][s][:], in_=NG16[b][:, s0:s1])
                    nc.vector.max_index(out=ix8[b][:, 8 * s : 8 * s + 8],
                                        in_max=mx8[b][s][:],
                                        in_values=NG16[b][:, s0:s1])
                nc.sync.dma_start(out=ixo[b], in_=ix8[b][:])

    nc.finalize()
    return nc


def _prep_host(pred_logits, anchors, target_boxes, target_present):
    f32, f16 = np.float32, np.float16
    A = np.ascontiguousarray(anchors.reshape(O, QP, 6).astype(f32, copy=False))
    pad = lambda x: np.pad(x, ((0, 0), (0, NCH * N - QP)), mode="edge")

    rc = [np.maximum(A[:, :, d], f32(0)) for d in range(3)]
    rsz = [np.maximum(A[:, :, 3 + d], f32(0)) for d in range(3)]
    alt = [pad(rc[d] - f32(0.5) * rsz[d]) for d in range(3)]
    arb = [pad(rc[d] + f32(0.5) * rsz[d]) for d in range(3)]
    apc = [pad(A[:, :, d]) for d in range(3)]
    planes = [alt[0], arb[0], alt[1], arb[1], alt[2], arb[2]] + apc
    ath = np.stack([p.reshape(P, N) for p in planes]).astype(f16)
    ath = np.ascontiguousarray(ath)

    lgs = pred_logits.reshape(BS, O, QP).astype(f32, copy=False)
    lgs = np.pad(lgs, ((0, 0), (0, 0), (0, NCH * N - QP)), mode="edge")
    lg_all = np.ascontiguousarray(lgs.reshape(BS, P, N).astype(f16))

    t = target_boxes.astype(f32, copy=False)          # [BS, O, 6]
    tc_, ts_ = t[..., :3], t[..., 3:]
    blt = tc_ - f32(0.5) * ts_
    brb = tc_ + f32(0.5) * ts_
    fd = brb - blt
    volb = (fd[..., 0] * fd[..., 1]) * fd[..., 2]

    in_maps = []
    for c in range(NCORES):
        b0 = c * BL
        scv = np.zeros((BL, P, NSC), f32)
        sc3 = scv.reshape(BL, O, NCH, NSC)
        for b in range(BL):
            gb = b0 + b
            sc3[b, :, :, C_BLT : C_BLT + 3] = blt[gb][:, None, :]
            sc3[b, :, :, C_BRB : C_BRB + 3] = brb[gb][:, None, :]
            sc3[b, :, :, C_FD : C_FD + 3] = fd[gb][:, None, :]
            sc3[b, :, :, C_VOLB] = volb[gb][:, None]
            sc3[b, :, :, C_TQ : C_TQ + 6] = f32(2.5) * t[gb][:, None, :]
        in_maps.append({"ath": ath,
                        "lg": np.ascontiguousarray(lg_all[b0 : b0 + BL]),
                        "sc": scv})
    return in_maps


def _host_post(res_results, pred_logits, anchors, target_boxes,
               target_present):
    """Exact matches via f64 rescore of device candidates; labels from
    the fp16 frac planes."""
    f64 = np.float64
    A = anchors.astype(f64).reshape(O, QP, 6)
    pl = pred_logits.astype(f64).reshape(BS, O, QP)
    t = target_boxes.astype(f64)
    present = target_present.astype(bool)

    # candidate q indices: per (b, organ, chunk) two 8-cand slices with
    # free-dim offsets 0 and 512
    K = NCH * 16
    cand = np.empty((BS, O, K), np.int64)
    frac = np.empty((BS, O, QP), np.float16)
    soff = np.array([s0 for s0, _ in MXS], np.int64)
    for c, r in enumerate(res_results):
        b0 = c * BL
        ix = r["ixo"].astype(np.int64).reshape(BL, O, NCH, 2, 8)
        gq = (ix + soff[None, None, None, :, None]
              + (np.arange(NCH, dtype=np.int64) * N)[None, None, :, None,
                                                     None])
        np.clip(gq, 0, QP - 1, out=gq)
        cand[b0 : b0 + BL] = gq.reshape(BL, O, K)
        frac[b0 : b0 + BL] = r["fr"].reshape(BL, O, NCH * N)[:, :, :QP]

    # f64 rescore with the exact reference formula
    bidx = np.arange(BS)[:, None, None]
    oidx = np.arange(O)[None, :, None]
    ab = A[oidx, cand]                                # [BS, O, K, 6]
    lgc = pl[bidx, oidx, cand]                        # [BS, O, K]
    tgt = t[:, :, None, :]                            # [BS, O, 1, 6]
    cb = np.abs(ab - tgt).sum(-1)
    cc = -1.0 / (1.0 + np.exp(-lgc))
    abx = np.clip(ab, 0.0, None)
    a_lt = abx[..., :3] - 0.5 * abx[..., 3:]
    a_rb = abx[..., :3] + 0.5 * abx[..., 3:]
    b_lt = tgt[..., :3] - 0.5 * tgt[..., 3:]
    b_rb = tgt[..., :3] + 0.5 * tgt[..., 3:]
    va = np.prod(a_rb - a_lt, -1)
    vb = np.prod(b_rb - b_lt, -1)
    it = np.prod(np.clip(np.minimum(a_rb, b_rb) - np.maximum(a_lt, b_lt),
                         0.0, None), -1)
    un = va + vb - it
    vcb = np.prod(np.clip(np.maximum(a_rb, b_rb) - np.minimum(a_lt, b_lt),
                          0.0, None), -1)
    giou = it / un - (vcb - un) / vcb
    Cc = 5.0 * cb + 2.0 * cc + 2.0 * (-giou)
    # argmin with lowest-q tie-break (reference top_k picks first index)
    order = np.lexsort((cand, Cc), axis=-1)
    best = np.take_along_axis(cand, order[..., :1], axis=-1)[..., 0]

    matches = np.zeros((BS, O, QP), np.int32)
    bo_b, bo_o = np.nonzero(present)
    matches[bo_b, bo_o, best[bo_b, bo_o]] = 1

    f = frac.astype(f64)
    fmin = f.min(-1, keepdims=True)
    fmax = f.max(-1, keepdims=True)
    sl = np.clip((f - fmin) / (fmax - fmin), 0.0, None).astype(np.float32)
    soft = np.where(present[..., None], sl, np.float32(-1.0))
    return matches, soft


def kernel(pred_logits, pred_boxes, anchors, target_boxes, target_present,
           num_top_queries):
    k = int(num_top_queries)
    assert k == 1, f"kernel specialized for num_top_queries=1, got {k}"

    if "nc" not in _BUILT:
        _BUILT["nc"] = _build_nc()
    nc = _BUILT["nc"]

    pred_logits = np.asarray(pred_logits)
    anchors = np.asarray(anchors)
    target_boxes = np.asarray(target_boxes)
    target_present = np.asarray(target_present)
    in_maps = _prep_host(pred_logits, anchors, target_boxes, target_present)
    res = run_bass_kernel_spmd(nc, in_maps, core_ids=list(range(NCORES)))
    return _host_post(res.results, pred_logits, anchors, target_boxes,
                      target_present)


# revision 36
# speedup vs baseline: 1.0381x; 1.0255x over previous
"""Trainium2 Bass kernel for the anchor-based NMS matcher (fp16 pipeline).

Math (see problem reference): per (batch b, organ o), over Qp=8192 anchor
queries q:
    cost_class = -sigmoid(logit)
    cost_bbox  = sum_d |anchor_d - tgt_d|            (cxcyczwhd space)
    cost_giou  = -giou3d(xyzxyz(clip(anchor,0)), xyzxyz(tgt))
    C = 5*cb + 2*cc + 2*cg
    matches     = one_hot(argmin_q C) * present
    soft_labels = present ? clip((cg-cgmax)/(cgmin-cgmax), 0) : -1

Device strategy (8 cores, data-parallel over batch, 2 batch items/core):
  SBUF layout: 120 partitions = (organ 20) x (q-chunk 6), free dim N=1366
  (6*1366 = 8196 = 8192 + 4 edge-pad).  All big planes are fp16: DVE
  tensor_scalar runs in 4x mode (416ns/plane) and tensor_tensor in 2x
  mode (772ns) vs 1483ns at fp32, and DMA bytes halve.  Only 9 anchor
  planes are DMA'd (alt/arb endpoints + 2.5x-scaled centers); sizes
  rs = arb - alt, vola, and the scaled size planes are derived on device
  during the DMA-bound startup window.

  negc = sigmoid - 2.5*cb + frac with frac = union/volc + inter/union
  (= giou + 1; affine-invariant for ranking and labels).  Work spread:
    DVE : interval endpoints, geometry, products, frac tail, top-8
    Act : sigmoid, |2.5(a-t)| via Abs, both giou reciprocals as
          exp(-ln(x)) with f32 intermediates (batched Ln then Exp to
          bound act-table reloads), PSUM->SBUF copies
    Pool: size planes, batch-1 p planes, vc2
    PE  : the whole negc sum tree as PSUM-accumulated +/-identity
          matmuls (sig + frac - ab0..ab5), fp16 weights, f32 accum
  Top-8 runs per half-row (copies pipeline into Max), so the device
  emits 16 candidate indices per (b, partition) plus the raw fp16 frac
  plane.  Host (numpy, f64) rescores the 96 candidates per (b,o) with
  the exact reference formula for exact argmin matches, and computes
  soft labels as the affine normalization of frac (absent organs
  patched to -1 on host).  Validated on the seeded data: full candidate
  coverage, label l2 rel err ~7e-4.
"""

import numpy as np

import concourse.bacc as bacc
import concourse.bass as bass
import concourse.mybir as mybir
from concourse.bass_utils import run_bass_kernel_spmd
from concourse.tile import TileContext

F32 = mybir.dt.float32
F16 = mybir.dt.float16
U16 = mybir.dt.uint16
U32 = mybir.dt.uint32
ALU = mybir.AluOpType
ACTF = mybir.ActivationFunctionType
AXL = mybir.AxisListType

BS, O, QP = 16, 20, 8192
NCORES = 8
BL = BS // NCORES        # batch items per core
NCH = 6                  # q chunks per organ
N = 1366                 # chunk width; 6*1366 = 8196 = 8192 + 4 pad
P = O * NCH              # 120 partitions
NPLANES = 9              # alt0,arb0,alt1,arb1,alt2,arb2, ap0-2 (2.5x ctr)
MMW = 512                # matmul moving free-dim chunk
MMC = [(0, 512), (512, 1024), (1024, 1366)]
MXS = [(0, 512), (512, 1366)]   # Max/MaxIndex slices

# sc column indices (per-partition f32 scalars, per batch item)
C_BLT = 0    # 0..2
C_BRB = 3    # 3..5
C_FD = 6     # 6..8
C_VOLB = 9
C_TQ = 10    # 10..15  (2.5 * target comps)
NSC = 16

_BUILT = {}


def _build_nc():
    nc = bacc.Bacc("TRN2", target_bir_lowering=False, debug=False)
    ath = nc.dram_tensor("ath", [NPLANES, P, N], F16, kind="ExternalInput")
    lg = nc.dram_tensor("lg", [BL, P, N], F16, kind="ExternalInput")
    sc = nc.dram_tensor("sc", [BL, P, NSC], F32, kind="ExternalInput")
    fr = nc.dram_tensor("fr", [BL, P, N], F16, kind="ExternalOutput")
    ixo = nc.dram_tensor("ixo", [BL, P, 16], U32, kind="ExternalOutput")

    from concourse.masks import make_identity

    with TileContext(nc) as tc:
        with (
            tc.tile_pool(name="big", bufs=1) as big,
            tc.tile_pool(name="sm", bufs=1) as sm,
            tc.tile_pool(name="ps", bufs=1, space="PSUM") as ps,
        ):
            # ---------------- small consts ----------------
            sct = sm.tile([P, BL, NSC], F32, tag="sct", name="sct")
            nc.scalar.dma_start(out=sct[:], in_=sc.rearrange("b p i -> p b i"))

            def col(b, i):
                return sct[:, b, i : i + 1]

            ipos = sm.tile([P, P], F16, tag="ipos", name="ipos")
            make_identity(nc, ipos[:])
            ineg = sm.tile([P, P], F16, tag="ineg", name="ineg")
            nc.gpsimd.memset(ineg[:], 0.0)
            nc.gpsimd.affine_select(
                out=ineg[:], in_=ineg[:],
                compare_op=ALU.not_equal, fill=-1.0, base=0,
                pattern=[[-1, P]], channel_multiplier=1)

            # ---------------- big inputs ----------------
            lgt = [big.tile([P, N], F16, tag=f"lg{b}", name=f"lg{b}")
                   for b in range(BL)]
            ain = big.tile([P, NPLANES, N], F16, tag="ain", name="ain")

            def v(j):
                return ain[:, j, :]

            ALT = [v(0), v(2), v(4)]
            ARB = [v(1), v(3), v(5)]
            APC = [v(6), v(7), v(8)]          # 2.5x center comps

            def load_planes(j0, j1):
                nc.sync.dma_start(out=ain[:, j0:j1, :],
                                  in_=ath[j0:j1].rearrange("i p n -> p i n"))

            load_planes(0, 2)           # alt0, arb0
            for b in range(BL):
                nc.scalar.dma_start(out=lgt[b][:], in_=lg[b])
            load_planes(2, 4)           # alt1, arb1
            load_planes(4, 6)           # alt2, arb2
            load_planes(6, 7)           # ap0
            load_planes(7, 8)           # ap1
            load_planes(8, 9)           # ap2

            # ---------------- per-batch tiles ----------------
            def bt(tag, b, dt=F16):
                return big.tile([P, N], dt, tag=f"{tag}{b}", name=f"{tag}{b}")

            RSD = [big.tile([P, N], F16, tag=f"rs{d}", name=f"rs{d}")
                   for d in range(3)]
            VOLA = big.tile([P, N], F16, tag="vola", name="vola")
            SIG = [bt("sig", b) for b in range(BL)]
            MX = [[bt(f"mx{d}_", b) for d in range(3)] for b in range(BL)]
            MN = [[bt(f"mn{d}_", b) for d in range(3)] for b in range(BL)]
            PD = [[bt(f"pd{d}_", b) for d in range(3)] for b in range(BL)]
            SF = [[bt(f"sf{d}_", b) for d in range(3)] for b in range(BL)]
            MM = [[bt(f"mm{d}_", b) for d in range(3)] for b in range(BL)]
            INT = [bt("int", b) for b in range(BL)]
            VLC = [bt("vlc", b) for b in range(BL)]
            UNI = [bt("uni", b) for b in range(BL)]
            RU = [bt("ru", b) for b in range(BL)]
            RV = [bt("rv", b) for b in range(BL)]
            LND = big.tile([P, N, 4], F32, tag="lnd", name="lnd")
            FRC = [bt("frc", b) for b in range(BL)]
            NG16 = [bt("ng16", b) for b in range(BL)]
            # SBUF reuse: abs planes land in MX/MN (dead after p_d);
            # vc_d lands in SF (dead after vc); XT lands in MM[0].
            VC = SF
            AB = [[MX[b][0], MX[b][1], MX[b][2],
                   MN[b][0], MN[b][1], MN[b][2]] for b in range(BL)]
            XT = [MM[b][0] for b in range(BL)]
            NGP = [ps.tile([P, 3, MMW], F32, tag=f"ngp{b}", name=f"ngp{b}")
                   for b in range(BL)]

            def mm_acc(b, plane, wt, start, stop, chunks=MMC):
                for c0, c1 in chunks:
                    c = MMC.index((c0, c1))
                    nc.tensor.matmul(NGP[b][:, c, : c1 - c0], wt[:],
                                     plane[:, c0:c1], start=start, stop=stop)

            # ---------------- geometry (batch-staggered) ----------------
            # b0's full chain -> b0 recips -> b0 tail overlap b1's chain,
            # which runs on DVE behind b0 with Pool feeding rs/p/vc2.
            for d in range(3):
                nc.gpsimd.tensor_tensor(out=RSD[d][:], in0=ARB[d],
                                        in1=ALT[d], op=ALU.subtract)
            for b in range(BL):
                for d in range(3):
                    nc.gpsimd.tensor_tensor(out=PD[1][d][:], in0=MN[1][d][:],
                                            in1=MX[1][d][:], op=ALU.subtract) \
                        if False else None
            # --- b0 chain (DVE) ---
            for d in range(3):
                nc.vector.tensor_scalar_max(out=MX[0][d][:], in0=ALT[d],
                                            scalar1=col(0, C_BLT + d))
                nc.vector.tensor_scalar_min(out=MN[0][d][:], in0=ARB[d],
                                            scalar1=col(0, C_BRB + d))
                nc.vector.tensor_tensor(out=PD[0][d][:], in0=MN[0][d][:],
                                        in1=MX[0][d][:], op=ALU.subtract)
                if d == 0:
                    for b in range(BL):
                        nc.scalar.activation(SIG[b][:], lgt[b][:],
                                             ACTF.Sigmoid)
            # b1 endpoints on DVE, p_b1 on Pool (behind rs)
            for d in range(3):
                nc.vector.tensor_scalar_max(out=MX[1][d][:], in0=ALT[d],
                                            scalar1=col(1, C_BLT + d))
                nc.vector.tensor_scalar_min(out=MN[1][d][:], in0=ARB[d],
                                            scalar1=col(1, C_BRB + d))
                nc.gpsimd.tensor_tensor(out=PD[1][d][:], in0=MN[1][d][:],
                                        in1=MX[1][d][:], op=ALU.subtract)
            for b in range(BL):
                nc.scalar.activation(AB[b][0][:], APC[0], ACTF.Abs,
                                     bias=col(b, C_TQ), scale=-2.5)
            # b0: sf/vc/relu/products/union/volc
            for d in range(3):
                nc.vector.tensor_scalar_add(out=SF[0][d][:], in0=RSD[d][:],
                                            scalar1=col(0, C_FD + d))
                nc.vector.tensor_scalar_max(out=MM[0][d][:],
                                            in0=PD[0][d][:], scalar1=0.0)
                nc.vector.tensor_tensor(out=VC[0][d][:], in0=SF[0][d][:],
                                        in1=PD[0][d][:], op=ALU.subtract)
            nc.vector.tensor_tensor(out=XT[0][:], in0=MM[0][0][:],
                                    in1=MM[0][1][:], op=ALU.mult)
            nc.vector.tensor_tensor(out=VOLA[:], in0=RSD[0][:],
                                    in1=RSD[1][:], op=ALU.mult)
            nc.vector.tensor_tensor(out=INT[0][:], in0=XT[0][:],
                                    in1=MM[0][2][:], op=ALU.mult)
            nc.vector.tensor_tensor(out=VOLA[:], in0=VOLA[:],
                                    in1=RSD[2][:], op=ALU.mult)
            nc.vector.tensor_scalar_add(out=UNI[0][:], in0=VOLA[:],
                                        scalar1=col(0, C_VOLB))
            nc.vector.tensor_tensor(out=UNI[0][:], in0=UNI[0][:],
                                    in1=INT[0][:], op=ALU.subtract)
            nc.vector.tensor_tensor(out=VLC[0][:], in0=VC[0][0][:],
                                    in1=VC[0][1][:], op=ALU.mult)
            nc.vector.tensor_tensor(out=VLC[0][:], in0=VLC[0][:],
                                    in1=VC[0][2][:], op=ALU.mult)
            for b in range(BL):
                nc.scalar.activation(AB[b][1][:], APC[1], ACTF.Abs,
                                     bias=col(b, C_TQ + 1), scale=-2.5)
                nc.scalar.activation(AB[b][4][:], RSD[1][:], ACTF.Abs,
                                     bias=col(b, C_TQ + 4), scale=-2.5)
                nc.scalar.activation(AB[b][5][:], RSD[2][:], ACTF.Abs,
                                     bias=col(b, C_TQ + 5), scale=-2.5)
            for b in range(BL):
                mm_acc(b, SIG[b], ipos, True, False)
                mm_acc(b, AB[b][0], ineg, False, False)
                mm_acc(b, AB[b][1], ineg, False, False)
                mm_acc(b, AB[b][4], ineg, False, False)
                mm_acc(b, AB[b][5], ineg, False, False)
            # --- b0 recips (Act): ln pair then exp pair, 2 loads ---
            nc.scalar.activation(LND[:, :, 0], UNI[0][:], ACTF.Ln)
            nc.scalar.activation(LND[:, :, 2], VLC[0][:], ACTF.Ln)
            nc.scalar.activation(RU[0][:], LND[:, :, 0], ACTF.Exp, scale=-1.0)
            nc.scalar.activation(RV[0][:], LND[:, :, 2], ACTF.Exp, scale=-1.0)
            # --- b1 chain (DVE, runs while b0 recips on Act) ---
            for d in range(3):
                nc.vector.tensor_scalar_add(out=SF[1][d][:], in0=RSD[d][:],
                                            scalar1=col(1, C_FD + d))
                nc.vector.tensor_scalar_max(out=MM[1][d][:],
                                            in0=PD[1][d][:], scalar1=0.0)
            for d in range(2):
                nc.vector.tensor_tensor(out=VC[1][d][:], in0=SF[1][d][:],
                                        in1=PD[1][d][:], op=ALU.subtract)
            nc.gpsimd.tensor_tensor(out=VC[1][2][:], in0=SF[1][2][:],
                                    in1=PD[1][2][:], op=ALU.subtract)
            nc.vector.tensor_tensor(out=XT[1][:], in0=MM[1][0][:],
                                    in1=MM[1][1][:], op=ALU.mult)
            nc.vector.tensor_tensor(out=INT[1][:], in0=XT[1][:],
                                    in1=MM[1][2][:], op=ALU.mult)
            nc.vector.tensor_scalar_add(out=UNI[1][:], in0=VOLA[:],
                                        scalar1=col(1, C_VOLB))
            nc.vector.tensor_tensor(out=UNI[1][:], in0=UNI[1][:],
                                    in1=INT[1][:], op=ALU.subtract)
            nc.vector.tensor_tensor(out=VLC[1][:], in0=VC[1][0][:],
                                    in1=VC[1][1][:], op=ALU.mult)
            nc.vector.tensor_tensor(out=VLC[1][:], in0=VLC[1][:],
                                    in1=VC[1][2][:], op=ALU.mult)
            # --- b1 recips ---
            nc.scalar.activation(LND[:, :, 1], UNI[1][:], ACTF.Ln)
            nc.scalar.activation(LND[:, :, 3], VLC[1][:], ACTF.Ln)
            nc.scalar.activation(RU[1][:], LND[:, :, 1], ACTF.Exp, scale=-1.0)
            nc.scalar.activation(RV[1][:], LND[:, :, 3], ACTF.Exp, scale=-1.0)
            # --- DVE abs pairs for ab2/ab3 (fill recip-wait holes) ---
            for b in range(BL):
                nc.vector.tensor_scalar(out=AB[b][2][:], in0=APC[2],
                                        scalar1=2.5, scalar2=col(b, C_TQ + 2),
                                        op0=ALU.mult, op1=ALU.subtract)
                nc.vector.tensor_scalar(out=AB[b][2][:].bitcast(U16),
                                        in0=AB[b][2][:].bitcast(U16),
                                        scalar1=0x7FFF, scalar2=None,
                                        op0=ALU.bitwise_and)
                nc.vector.tensor_scalar(out=AB[b][3][:], in0=RSD[0][:],
                                        scalar1=2.5, scalar2=col(b, C_TQ + 3),
                                        op0=ALU.mult, op1=ALU.subtract)
                nc.vector.tensor_scalar(out=AB[b][3][:].bitcast(U16),
                                        in0=AB[b][3][:].bitcast(U16),
                                        scalar1=0x7FFF, scalar2=None,
                                        op0=ALU.bitwise_and)
            for b in range(BL):
                mm_acc(b, AB[b][2], ineg, False, False)
                mm_acc(b, AB[b][3], ineg, False, False)
            # --- tails ---
            mx8 = [[sm.tile([P, 8], F16, tag=f"mx8_{b}_{s}",
                            name=f"mx8_{b}_{s}") for s in range(2)]
                   for b in range(BL)]
            ix8 = [sm.tile([P, 16], U32, tag=f"ix8_{b}", name=f"ix8_{b}")
                   for b in range(BL)]
            for b in range(BL):
                nc.vector.tensor_tensor(out=XT[b][:], in0=UNI[b][:],
                                        in1=RV[b][:], op=ALU.mult)
                nc.vector.tensor_tensor(out=FRC[b][:], in0=INT[b][:],
                                        in1=RU[b][:], op=ALU.mult)
                nc.vector.tensor_tensor(out=FRC[b][:], in0=FRC[b][:],
                                        in1=XT[b][:], op=ALU.add)
                nc.sync.dma_start(out=fr[b], in_=FRC[b][:])
                mm_acc(b, FRC[b], ipos, False, True)
            for b in range(BL):
                for c, (c0, c1) in enumerate(MMC):
                    nc.scalar.activation(NG16[b][:, c0:c1],
                                         NGP[b][:, c, : c1 - c0], ACTF.Copy)
                for s, (s0, s1) in enumerate(MXS):
                    nc.vector.max(out=mx8[b][s][:], in_=NG16[b][:, s0:s1])
                    nc.vector.max_index(out=ix8[b][:, 8 * s : 8 * s + 8],
                                        in_max=mx8[b][s][:],
                                        in_values=NG16[b][:, s0:s1])
                nc.sync.dma_start(out=ixo[b], in_=ix8[b][:])

    nc.finalize()
    return nc


def _prep_host(pred_logits, anchors, target_boxes, target_present):
    f32, f16 = np.float32, np.float16
    A = np.ascontiguousarray(anchors.reshape(O, QP, 6).astype(f32, copy=False))
    pad = lambda x: np.pad(x, ((0, 0), (0, NCH * N - QP)), mode="edge")

    rc = [np.maximum(A[:, :, d], f32(0)) for d in range(3)]
    rsz = [np.maximum(A[:, :, 3 + d], f32(0)) for d in range(3)]
    alt = [pad(rc[d] - f32(0.5) * rsz[d]) for d in range(3)]
    arb = [pad(rc[d] + f32(0.5) * rsz[d]) for d in range(3)]
    apc = [pad(A[:, :, d]) for d in range(3)]
    planes = [alt[0], arb[0], alt[1], arb[1], alt[2], arb[2]] + apc
    ath = np.stack([p.reshape(P, N) for p in planes]).astype(f16)
    ath = np.ascontiguousarray(ath)

    lgs = pred_logits.reshape(BS, O, QP).astype(f32, copy=False)
    lgs = np.pad(lgs, ((0, 0), (0, 0), (0, NCH * N - QP)), mode="edge")
    lg_all = np.ascontiguousarray(lgs.reshape(BS, P, N).astype(f16))

    t = target_boxes.astype(f32, copy=False)          # [BS, O, 6]
    tc_, ts_ = t[..., :3], t[..., 3:]
    blt = tc_ - f32(0.5) * ts_
    brb = tc_ + f32(0.5) * ts_
    fd = brb - blt
    volb = (fd[..., 0] * fd[..., 1]) * fd[..., 2]

    in_maps = []
    for c in range(NCORES):
        b0 = c * BL
        scv = np.zeros((BL, P, NSC), f32)
        sc3 = scv.reshape(BL, O, NCH, NSC)
        for b in range(BL):
            gb = b0 + b
            sc3[b, :, :, C_BLT : C_BLT + 3] = blt[gb][:, None, :]
            sc3[b, :, :, C_BRB : C_BRB + 3] = brb[gb][:, None, :]
            sc3[b, :, :, C_FD : C_FD + 3] = fd[gb][:, None, :]
            sc3[b, :, :, C_VOLB] = volb[gb][:, None]
            sc3[b, :, :, C_TQ : C_TQ + 6] = f32(2.5) * t[gb][:, None, :]
        in_maps.append({"ath": ath,
                        "lg": np.ascontiguousarray(lg_all[b0 : b0 + BL]),
                        "sc": scv})
    return in_maps


def _host_post(res_results, pred_logits, anchors, target_boxes,
               target_present):
    """Exact matches via f64 rescore of device candidates; labels from
    the fp16 frac planes."""
    f64 = np.float64
    A = anchors.astype(f64).reshape(O, QP, 6)
    pl = pred_logits.astype(f64).reshape(BS, O, QP)
    t = target_boxes.astype(f64)
    present = target_present.astype(bool)

    # candidate q indices: per (b, organ, chunk) two 8-cand slices with
    # free-dim offsets 0 and 512
    K = NCH * 16
    cand = np.empty((BS, O, K), np.int64)
    frac = np.empty((BS, O, QP), np.float16)
    soff = np.array([s0 for s0, _ in MXS], np.int64)
    for c, r in enumerate(res_results):
        b0 = c * BL
        ix = r["ixo"].astype(np.int64).reshape(BL, O, NCH, 2, 8)
        gq = (ix + soff[None, None, None, :, None]
              + (np.arange(NCH, dtype=np.int64) * N)[None, None, :, None,
                                                     None])
        np.clip(gq, 0, QP - 1, out=gq)
        cand[b0 : b0 + BL] = gq.reshape(BL, O, K)
        frac[b0 : b0 + BL] = r["fr"].reshape(BL, O, NCH * N)[:, :, :QP]

    # f64 rescore with the exact reference formula
    bidx = np.arange(BS)[:, None, None]
    oidx = np.arange(O)[None, :, None]
    ab = A[oidx, cand]                                # [BS, O, K, 6]
    lgc = pl[bidx, oidx, cand]                        # [BS, O, K]
    tgt = t[:, :, None, :]                            # [BS, O, 1, 6]
    cb = np.abs(ab - tgt).sum(-1)
    cc = -1.0 / (1.0 + np.exp(-lgc))
    abx = np.clip(ab, 0.0, None)
    a_lt = abx[..., :3] - 0.5 * abx[..., 3:]
    a_rb = abx[..., :3] + 0.5 * abx[..., 3:]
    b_lt = tgt[..., :3] - 0.5 * tgt[..., 3:]
    b_rb = tgt[..., :3] + 0.5 * tgt[..., 3:]
    va = np.prod(a_rb - a_lt, -1)
    vb = np.prod(b_rb - b_lt, -1)
    it = np.prod(np.clip(np.minimum(a_rb, b_rb) - np.maximum(a_lt, b_lt),
                         0.0, None), -1)
    un = va + vb - it
    vcb = np.prod(np.clip(np.maximum(a_rb, b_rb) - np.minimum(a_lt, b_lt),
                          0.0, None), -1)
    giou = it / un - (vcb - un) / vcb
    Cc = 5.0 * cb + 2.0 * cc + 2.0 * (-giou)
    # argmin with lowest-q tie-break (reference top_k picks first index)
    order = np.lexsort((cand, Cc), axis=-1)
    best = np.take_along_axis(cand, order[..., :1], axis=-1)[..., 0]

    matches = np.zeros((BS, O, QP), np.int32)
    bo_b, bo_o = np.nonzero(present)
    matches[bo_b, bo_o, best[bo_b, bo_o]] = 1

    f = frac.astype(f64)
    fmin = f.min(-1, keepdims=True)
    fmax = f.max(-1, keepdims=True)
    sl = np.clip((f - fmin) / (fmax - fmin), 0.0, None).astype(np.float32)
    soft = np.where(present[..., None], sl, np.float32(-1.0))
    return matches, soft


def kernel(pred_logits, pred_boxes, anchors, target_boxes, target_present,
           num_top_queries):
    k = int(num_top_queries)
    assert k == 1, f"kernel specialized for num_top_queries=1, got {k}"

    if "nc" not in _BUILT:
        _BUILT["nc"] = _build_nc()
    nc = _BUILT["nc"]

    pred_logits = np.asarray(pred_logits)
    anchors = np.asarray(anchors)
    target_boxes = np.asarray(target_boxes)
    target_present = np.asarray(target_present)
    in_maps = _prep_host(pred_logits, anchors, target_boxes, target_present)
    res = run_bass_kernel_spmd(nc, in_maps, core_ids=list(range(NCORES)))
    return _host_post(res.results, pred_logits, anchors, target_boxes,
                      target_present)
